# revision 33
# baseline (speedup 1.0000x reference)
"""Trainium2 Bass kernel for the additive-attention transformer.

Sharding: 8 cores = (batch b in 0..3) x (sequence half in 0..1).
Each core owns 128 query rows of one batch through 3 encoder layers.

Exchange: after layers 0 and 1, core pairs AllGather [z_bf16 | kT_next]
(1408 cols).  Each core's key/value tiles are ordered own-half-first by
the host, so the SPMD program never needs its rank; peer data is pulled
from the gathered buffer with one indirect DMA using a host-provided
row-index tensor.  Own-half attention (fpre/tanh/scores/exp/partial
softmax-sums/partial attnV) overlaps the collective.

The next layer's k/q projections never transpose z: by linearity
W'z^T = W'(ybt + o2t) where ybt (FFN input transposes) and o2t
(W2^T @ h1, straight off the PE) already exist, and the constant b2
term folds into the q-side as a per-partition bias (delta = W'^T b2).
This removes 10 PE transposes + copies per layer, starts the exchange
earlier, and computes the projections in f32 PSUM (better precision
than the old bf16-z transpose path).

The tiny layer-4 attention (one query row per batch) and the 3-matmul
head run on the host in fp32.
"""

import numpy as np
import ml_dtypes

import concourse.bass as bass
import concourse.mybir as mybir
import concourse.tile as tile
from concourse import bacc
from concourse.bass_utils import run_bass_kernel_spmd
from concourse.masks import make_identity

F32 = mybir.dt.float32
BF16 = mybir.dt.bfloat16
I32 = mybir.dt.int32
AF = mybir.ActivationFunctionType
ALU = mybir.AluOpType

V, H, B, S = 1280, 128, 4, 256
P = 128          # partitions / own rows per core
VC = V // P      # 10 v-chunks
NCORES = 8
AG = V + H       # exchange payload cols
EPS = 1e-5

_CACHE = {}


def _build():
    nc = bacc.Bacc("TRN2", target_bir_lowering=False, debug=False,
                   num_devices=NCORES)

    # ---- I/O ----
    x32_in = nc.dram_tensor("x32", [P, V], F32, kind="ExternalInput")
    xot_in = nc.dram_tensor("xot", [P, VC, P], BF16, kind="ExternalInput")
    xft_in = nc.dram_tensor("xft", [P, VC, S], BF16, kind="ExternalInput")
    va_in = nc.dram_tensor("va", [P, V], BF16, kind="ExternalInput")
    vb_in = nc.dram_tensor("vb", [P, V], BF16, kind="ExternalInput")
    idx_in = nc.dram_tensor("idxp", [P, 1], I32, kind="ExternalInput")
    w_in = {}
    for l in range(3):
        w_in[f"wq{l}"] = nc.dram_tensor(f"wq{l}", [P, VC, H], BF16, kind="ExternalInput")
        w_in[f"wk{l}"] = nc.dram_tensor(f"wk{l}", [P, VC, H], BF16, kind="ExternalInput")
        w_in[f"wv{l}"] = nc.dram_tensor(f"wv{l}", [P, 1], BF16, kind="ExternalInput")
        w_in[f"w1{l}"] = nc.dram_tensor(f"w1{l}", [P, VC, H], BF16, kind="ExternalInput")
        w_in[f"b1{l}"] = nc.dram_tensor(f"b1{l}", [P, 1], F32, kind="ExternalInput")
        w_in[f"w2{l}"] = nc.dram_tensor(f"w2{l}", [P, V], BF16, kind="ExternalInput")
        w_in[f"b2{l}"] = nc.dram_tensor(f"b2{l}", [1, V], BF16, kind="ExternalInput")
        w_in[f"b2c{l}"] = nc.dram_tensor(f"b2c{l}", [P, VC], BF16, kind="ExternalInput")
    zout = nc.dram_tensor("zout", [P, V], F32, kind="ExternalOutput")

    # AllGather bounce buffers (after layers 0, 1); agout viewed [2P, AG]
    agin = [nc.dram_tensor(f"agin{l}", [P, AG], BF16) for l in range(2)]
    agout = [nc.dram_tensor(f"agout{l}", [2 * P, AG], BF16) for l in range(2)]
    groups = [[0, 1], [2, 3], [4, 5], [6, 7]]

    with tile.TileContext(nc) as tc:
        with tc.tile_pool(name="persist", bufs=1) as pp, \
             tc.tile_pool(name="xbuf", bufs=2) as xb, \
             tc.tile_pool(name="scratch", bufs=2) as sc, \
             tc.tile_pool(name="feat", bufs=2) as fp, \
             tc.tile_pool(name="ps", bufs=1, space="PSUM") as ps, \
             tc.tile_pool(name="ps2", bufs=2, space="PSUM") as ps2:

            ident = pp.tile([P, P], BF16, tag="ident")
            make_identity(nc, ident[:])
            ones = pp.tile([P, 1], BF16, tag="ones")
            nc.vector.memset(ones[:], 1.0)
            onesr = pp.tile([1, P], BF16, tag="onesr")
            nc.vector.memset(onesr[:], 1.0)

            # ---- input loads (layer-0 critical path first) ----
            w = {}

            def _load_w(k):
                t = w_in[k]
                tl = pp.tile(list(t.shape), t.dtype, tag=k)
                nc.sync.dma_start(tl[:], t[(slice(None),) * len(t.shape)])
                w[k] = tl

            xft = pp.tile([P, VC, S], BF16, tag="xft")
            nc.sync.dma_start(xft[:], xft_in[:, :, :])
            _load_w("wk0")
            xot = pp.tile([P, VC, P], BF16, tag="xot")
            nc.sync.dma_start(xot[:], xot_in[:, :, :])
            _load_w("wq0")
            _load_w("wv0")
            va0 = pp.tile([P, V], BF16, tag="va0")
            nc.sync.dma_start(va0[:], va_in[:, :])
            vb0 = pp.tile([P, V], BF16, tag="vb0")
            nc.sync.dma_start(vb0[:], vb_in[:, :])
            x32 = xb.tile([P, V], F32, tag="z32")
            nc.sync.dma_start(x32[:], x32_in[:, :])
            idxt = pp.tile([P, 1], I32, tag="idxp")
            nc.sync.dma_start(idxt[:], idx_in[:, :])

            for l in range(3):
                for base in ("wq", "wk", "wv", "w1", "b1", "w2", "b2", "b2c"):
                    k = f"{base}{l}"
                    if k not in w:
                        _load_w(k)

            # ---- layer-0 k projection over both halves (own-first order) ----
            kt_ps = ps.tile([P, S], F32, tag="pk")
            for c in range(VC):
                nc.tensor.matmul(kt_ps[:], w["wk0"][:, c, :], xft[:, c, :],
                                 start=(c == 0), stop=(c == VC - 1))
            kts0 = pp.tile([P, S], BF16, tag="kts0")
            nc.vector.tensor_copy(kts0[:], kt_ps[:])

            # layer-0 q projection (later layers: computed in previous tail)
            qt_ps0 = ps.tile([P, P], F32, tag="pk", name="qt0")
            for c in range(VC):
                nc.tensor.matmul(qt_ps0[:], w["wq0"][:, c, :], xot[:, c, :],
                                 start=(c == 0), stop=(c == VC - 1))
            qts = sc.tile([P, P], F32, tag="qts", name="qts0")
            nc.vector.tensor_copy(qts[:], qt_ps0[:])

            # layer state (python vars pointing at tiles)
            z32 = x32                 # own rows, natural, f32 residual
            ka = kts0[:, 0:P]         # own-half kT [h, 128]
            kb = kts0[:, P:S]         # peer-half kT
            va = va0                  # own-half values [j, V]
            vb = vb0                  # peer-half values

            BI = 32                   # max query rows per feat block

            for l in range(3):
                # ---- feat/scores/exp/sums/attnV per half (own first) ----
                sums = ps.tile([P, 1], F32, tag="pk", name=f"sums{l}")
                av = ps.tile([P, V], F32, tag="big", name=f"av{l}")
                scta = ps.tile([P, P], F32, tag="scta", name=f"scta{l}")
                sctb = ps.tile([P, P], F32, tag="sctb", name=f"sctb{l}")
                scts = (scta, sctb)

                def _softmax_av(seg, sct, vh):
                    expt = sc.tile([P, P], BF16, tag=("expa", "expb")[seg],
                                   name=f"exp{seg}_{l}")
                    nc.scalar.activation(out=expt[:], in_=sct[:], func=AF.Exp)
                    nc.tensor.matmul(sums[:], expt[:], ones[:],
                                     start=(seg == 0), stop=(seg == 1))
                    for off in range(0, V, 512):
                        n = min(512, V - off)
                        nc.tensor.matmul(av[:, off:off + n], expt[:],
                                         vh[:, off:off + n],
                                         start=(seg == 0), stop=(seg == 1))

                if l == 0:
                    # both halves local: full-S fpre rows (fewer DVE ops)
                    for blk in range(8):
                        fpre = fp.tile([P, 16, S], BF16, tag="fpre")
                        for ii in range(16):
                            i = blk * 16 + ii
                            nc.vector.tensor_scalar(
                                out=fpre[:, ii, :], in0=kts0[:],
                                scalar1=qts[:, i:i + 1],
                                scalar2=None, op0=ALU.add)
                        feat = fp.tile([P, 16, S], BF16, tag="feat")
                        nc.scalar.activation(out=feat[:], in_=fpre[:],
                                             func=AF.Tanh)
                        for ii in range(16):
                            i = blk * 16 + ii
                            for seg in range(2):
                                nc.tensor.matmul(
                                    scts[seg][:, i:i + 1],
                                    feat[:, ii, seg * P:(seg + 1) * P],
                                    w[f"wv{l}"][:], start=True, stop=True)
                    _softmax_av(0, scta, va)
                    _softmax_av(1, sctb, vb)
                else:
                    for seg, (kth, vh) in enumerate(((ka, va), (kb, vb))):
                        sct = scts[seg]
                        i = 0
                        for bi in (16, 32, 32, 32, 16):
                            fpre = fp.tile([P, BI, P], BF16, tag="fpre")
                            for ii in range(bi):
                                nc.vector.tensor_scalar(
                                    out=fpre[:, ii, :], in0=kth,
                                    scalar1=qts[:, i + ii:i + ii + 1],
                                    scalar2=None, op0=ALU.add)
                            feat = fp.tile([P, BI, P], BF16, tag="feat")
                            nc.scalar.activation(out=feat[:, 0:bi, :],
                                                 in_=fpre[:, 0:bi, :],
                                                 func=AF.Tanh)
                            for ii in range(bi):
                                nc.tensor.matmul(sct[:, i + ii:i + ii + 1],
                                                 feat[:, ii, :],
                                                 w[f"wv{l}"][:],
                                                 start=True, stop=True)
                            i += bi
                        _softmax_av(seg, sct, vh)

                # ---- softmax normalize + residual + LN ----
                rin = sc.tile([P, 1], F32, tag="rin")
                nc.vector.reciprocal(rin[:], sums[:])
                ax = sc.tile([P, V], F32, tag="ax")
                nc.scalar.activation(out=ax[:], in_=av[:], func=AF.Copy,
                                     scale=rin[:])
                nc.vector.tensor_add(out=ax[:], in0=ax[:], in1=z32[:])

                stats = sc.tile([P, 5, 6], F32, tag="stats")
                axg = ax[:].rearrange("p (n s) -> p n s", s=256)
                for g in range(5):
                    nc.vector.bn_stats(out=stats[:, g, :], in_=axg[:, g, :])
                mv = sc.tile([P, 2], F32, tag="mv")
                nc.vector.bn_aggr(out=mv[:], in_=stats[:])
                # rstd = 1/sqrt(var+eps) via Newton on DVE (r0 from 1/v fit)
                vv = sc.tile([P, 1], F32, tag="vv")
                nc.vector.tensor_scalar(out=vv[:], in0=mv[:, 1:2], scalar1=EPS,
                                        scalar2=None, op0=ALU.add)
                s_ = sc.tile([P, 1], F32, tag="s_")
                nc.vector.reciprocal(s_[:], vv[:])
                r_ = sc.tile([P, 1], F32, tag="r_")
                nc.vector.tensor_scalar(out=r_[:], in0=s_[:], scalar1=0.4315,
                                        scalar2=0.361, op0=ALU.mult, op1=ALU.add)
                t1 = sc.tile([P, 1], F32, tag="t1")
                for _ in range(4):
                    nc.vector.tensor_mul(out=t1[:], in0=vv[:], in1=r_[:])
                    nc.vector.tensor_mul(out=t1[:], in0=t1[:], in1=r_[:])
                    nc.vector.tensor_scalar(out=t1[:], in0=t1[:], scalar1=-0.5,
                                            scalar2=1.5, op0=ALU.mult, op1=ALU.add)
                    nc.vector.tensor_mul(out=r_[:], in0=r_[:], in1=t1[:])
                yb = sc.tile([P, V], BF16, tag="yb")
                for g in range(5):
                    gs = slice(g * 256, (g + 1) * 256)
                    nc.vector.tensor_scalar(out=yb[:, gs], in0=ax[:, gs],
                                            scalar1=mv[:, 0:1], scalar2=r_[:],
                                            op0=ALU.subtract, op1=ALU.mult)
                mrn = sc.tile([P, 1], F32, tag="mrn")
                nc.vector.tensor_mul(out=mrn[:], in0=mv[:, 0:1], in1=r_[:])
                nc.vector.tensor_scalar(out=mrn[:], in0=mrn[:], scalar1=-1.0,
                                        scalar2=None, op0=ALU.mult)
                y32 = sc.tile([P, V], F32, tag="y32")
                nc.scalar.activation(out=y32[:], in_=ax[:], func=AF.Identity,
                                     scale=r_[:], bias=mrn[:])

                # ---- FFN with transpose interleaved per chunk ----
                ybt = sc.tile([P, VC, P], BF16, tag="ybt")
                h1_ps = ps.tile([P, P], F32, tag="scta", name=f"h1{l}")
                for c in range(VC):
                    yt_ps = ps2.tile([P, P], BF16, tag="yt")
                    nc.tensor.transpose(yt_ps[:], yb[:, c * P:(c + 1) * P], ident[:])
                    if c % 3 != 2:
                        nc.vector.tensor_copy(ybt[:, c, :], yt_ps[:])
                    else:
                        nc.scalar.copy(ybt[:, c, :], yt_ps[:])
                    nc.tensor.matmul(h1_ps[:], w[f"w1{l}"][:, c, :], ybt[:, c, :],
                                     start=(c == 0), stop=(c == VC - 1))
                h1r = sc.tile([P, P], BF16, tag="h1r")
                nc.scalar.activation(out=h1r[:], in_=h1_ps[:], func=AF.Relu,
                                     bias=w[f"b1{l}"][:], scale=1.0)
                o2 = ps.tile([P, V], F32, tag="big", name=f"o2{l}")
                for off in range(0, V, 512):
                    n = min(512, V - off)
                    nc.tensor.matmul(o2[:, off:off + n], onesr[:],
                                     w[f"b2{l}"][0:1, off:off + n],
                                     start=True, stop=False)
                for off in range(0, V, 512):
                    n = min(512, V - off)
                    nc.tensor.matmul(o2[:, off:off + n], h1r[:],
                                     w[f"w2{l}"][:, off:off + n],
                                     start=False, stop=True)
                z32n = xb.tile([P, V], F32, tag="z32")
                for g in range(5):
                    gs = slice(g * 256, (g + 1) * 256)
                    nc.vector.tensor_add(out=z32n[:, gs], in0=o2[:, gs],
                                         in1=y32[:, gs])

                if l == 2:
                    for g in range(5):
                        gs = slice(g * 256, (g + 1) * 256)
                        nc.sync.dma_start(zout[:, gs], z32n[:, gs])
                    break

                # ---- zb; next-layer kT/qT via linearity (z = o2 + y + b2:
                # W'z^T = W'(o2t + ybt) per chunk, b2 term folded into q bias)
                zb = xb.tile([P, V], BF16, tag="zb")
                for g in range(5):
                    gs = slice(g * 256, (g + 1) * 256)
                    nc.vector.tensor_copy(zb[:, gs], z32n[:, gs])
                nc.sync.dma_start(agin[l][:, 0:V], zb[:])

                o2ts = sc.tile([P, VC, P], BF16, tag="o2ts")
                for c in range(VC):
                    ot_ps = ps2.tile([P, P], F32, tag="yt")
                    nc.tensor.matmul(ot_ps[:], w[f"w2{l}"][:, c * P:(c + 1) * P],
                                     h1r[:], start=True, stop=True)
                    if c % 3 != 2:
                        nc.vector.tensor_copy(o2ts[:, c, :], ot_ps[:])
                    else:
                        nc.scalar.copy(o2ts[:, c, :], ot_ps[:])

                ktn_ps = ps.tile([P, P], F32, tag="pk", name=f"ktn{l}")
                for c in range(VC):
                    nc.tensor.matmul(ktn_ps[:], w[f"wk{l + 1}"][:, c, :],
                                     ybt[:, c, :],
                                     start=(c == 0), stop=False)
                for c in range(VC):
                    nc.tensor.matmul(ktn_ps[:], w[f"wk{l + 1}"][:, c, :],
                                     o2ts[:, c, :],
                                     start=False, stop=(c == VC - 1))
                kan = xb.tile([P, P], BF16, tag="ka")
                nc.vector.tensor_copy(kan[:], ktn_ps[:])

                nc.sync.dma_start(agin[l][:, V:AG], kan[:])
                nc.gpsimd.collective_compute(
                    "AllGather", ALU.bypass, replica_groups=groups,
                    ins=[agin[l][:, :]], outs=[agout[l][:, :]])

                # delta = W'^T b2 (q/k bias deficit of the linear path)
                dlt_ps = ps.tile([P, 2], F32, tag="pk", name=f"dlt{l}")
                for c in range(VC):
                    nc.tensor.matmul(dlt_ps[:, 0:1], w[f"wq{l + 1}"][:, c, :],
                                     w[f"b2c{l}"][:, c:c + 1],
                                     start=(c == 0), stop=(c == VC - 1))
                for c in range(VC):
                    nc.tensor.matmul(dlt_ps[:, 1:2], w[f"wk{l + 1}"][:, c, :],
                                     w[f"b2c{l}"][:, c:c + 1],
                                     start=(c == 0), stop=(c == VC - 1))
                dlt = sc.tile([P, 2], F32, tag="dlt")
                nc.vector.tensor_copy(dlt[:], dlt_ps[:])

                qtn_ps = ps.tile([P, P], F32, tag="pk", name=f"qt{l + 1}")
                for c in range(VC):
                    nc.tensor.matmul(qtn_ps[:], w[f"wq{l + 1}"][:, c, :],
                                     ybt[:, c, :],
                                     start=(c == 0), stop=False)
                for c in range(VC):
                    nc.tensor.matmul(qtn_ps[:], w[f"wq{l + 1}"][:, c, :],
                                     o2ts[:, c, :],
                                     start=False, stop=(c == VC - 1))
                qtsn = sc.tile([P, P], F32, tag="qts", name=f"qts{l + 1}")
                nc.vector.tensor_scalar(out=qtsn[:], in0=qtn_ps[:],
                                        scalar1=dlt[:, 0:1],
                                        scalar2=dlt[:, 1:2],
                                        op0=ALU.add, op1=ALU.add)

                vkt = xb.tile([P, AG], BF16, tag="vkt")
                nc.gpsimd.indirect_dma_start(
                    out=vkt[:], out_offset=None,
                    in_=agout[l][:, :],
                    in_offset=bass.IndirectOffsetOnAxis(ap=idxt[:, 0:1], axis=0))

                z32, qts = z32n, qtsn
                ka, kb = kan[:, :], vkt[:, V:AG]
                va, vb = zb, vkt[:, 0:V]

    nc.compile()
    return nc


def _bf(a):
    return np.ascontiguousarray(a.astype(ml_dtypes.bfloat16))


def kernel(**inputs):
    X = np.asarray(inputs["X"], dtype=np.float32)
    lys = int(np.asarray(inputs["lys_pos"]))
    if "nc" not in _CACHE:
        _CACHE["nc"] = _build()
    nc = _CACHE["nc"]

    # host-side prearranged shared (replicated) weights
    wshared = {}
    for l, li in enumerate((1, 2, 3)):
        Wq = np.asarray(inputs[f"Wq{li}"], np.float32)
        Wk = np.asarray(inputs[f"Wk{li}"], np.float32)
        W1 = np.asarray(inputs[f"rW1_{li}"], np.float32)
        W2 = np.asarray(inputs[f"rW2_{li}"], np.float32)
        wshared[f"wq{l}"] = _bf(Wq.reshape(VC, P, H).transpose(1, 0, 2))
        wshared[f"wk{l}"] = _bf(Wk.reshape(VC, P, H).transpose(1, 0, 2))
        wshared[f"wv{l}"] = _bf(np.asarray(inputs[f"wv{li}"], np.float32)[:, None])
        wshared[f"w1{l}"] = _bf(W1.reshape(VC, P, H).transpose(1, 0, 2))
        wshared[f"b1{l}"] = np.ascontiguousarray(
            np.asarray(inputs[f"rb1_{li}"], np.float32)[:, None])
        wshared[f"w2{l}"] = _bf(W2)
        b2v = np.asarray(inputs[f"rb2_{li}"], np.float32)
        wshared[f"b2{l}"] = _bf(b2v[None, :])
        wshared[f"b2c{l}"] = _bf(b2v.reshape(VC, P).T)

    in_maps = []
    for c in range(NCORES):
        b, h = c // 2, c % 2
        Xb = X[b]                            # [S, V]
        own = Xb[h * P:(h + 1) * P]          # [P, V]
        peer = Xb[(1 - h) * P:(2 - h) * P]   # [P, V]
        m = dict(wshared)
        m["x32"] = np.ascontiguousarray(own)
        m["xot"] = _bf(own.T.reshape(VC, P, P).transpose(1, 0, 2))
        # X^T with columns ordered own-half first
        Xo = np.concatenate([own, peer], axis=0)        # [S, V]
        m["xft"] = _bf(Xo.T.reshape(VC, P, S).transpose(1, 0, 2))
        m["va"] = _bf(own)
        m["vb"] = _bf(peer)
        m["idxp"] = np.ascontiguousarray(
            (np.arange(P, dtype=np.int32) + P * (1 - h))[:, None])
        in_maps.append(m)

    res = run_bass_kernel_spmd(nc, in_maps, core_ids=list(range(NCORES)))

    X3 = np.zeros((B, S, V), np.float32)
    for c in range(NCORES):
        b, h = c // 2, c % 2
        X3[b, h * P:(h + 1) * P] = res.results[c]["zout"]

    # ---- layer 4 + head on host (fp32) ----
    def ln(x):
        m_ = x.mean(-1, keepdims=True)
        v_ = ((x - m_) ** 2).mean(-1, keepdims=True)
        return (x - m_) / np.sqrt(v_ + EPS)

    Wq4 = np.asarray(inputs["Wq4"], np.float32)
    Wk4 = np.asarray(inputs["Wk4"], np.float32)
    wv4 = np.asarray(inputs["wv4"], np.float32)
    Xl = X3[:, lys, :][:, None, :]                       # [B,1,V]
    q = Xl @ Wq4                                         # [B,1,H]
    k = X3 @ Wk4                                         # [B,S,H]
    feat = np.tanh(q[:, :, None, :] + k[:, None, :, :])  # [B,1,S,H]
    sco = np.einsum("bijh,h->bij", feat, wv4)
    sco = sco - sco.max(-1, keepdims=True)
    a = np.exp(sco)
    a /= a.sum(-1, keepdims=True)
    att = np.einsum("bij,bjd->bid", a, X3)
    Xl = ln(att + Xl)
    h_ = np.maximum(Xl @ np.asarray(inputs["hW1"], np.float32)
                    + np.asarray(inputs["hb1"], np.float32), 0.0)
    h_ = np.maximum(h_ @ np.asarray(inputs["hW2"], np.float32)
                    + np.asarray(inputs["hb2"], np.float32), 0.0)
    logits = (h_ @ np.asarray(inputs["hW3"], np.float32)
              + np.asarray(inputs["hb3"], np.float32))[:, 0, :]
    return logits.astype(np.float32)
